# revision 12
# baseline (speedup 1.0000x reference)
"""CSSM TinyViT block on 8 TRN2 NeuronCores — fp8/bf16 rebalanced version.

Strategy
--------
Data-parallel over batch: B=16 -> 2 samples (2048 tokens) per core.

Measured on HW: fp8 matmuls stream 2x faster than f32r/bf16; the DVE is
the real bottleneck of the f32r design.  So:

- Projections (Wu, Wg), MLP (w1, w2): fp8-e4m3 weights (pre-scaled x64 on
  host, descaled in the PSUM-consuming op) with fp8 activations written
  directly by the Activation engine (LN apply / gelu emit fp8).
- Scan stays bf16 (fp8 state accumulates too much error): 7 steps of
  h <- g*(A^T h + md), with the constant md = (1/g-1)*(u+bu) folded into
  the PSUM accumulation via an extra eye-matmul, so each chunk-step costs
  ONE cheap elementwise op (Act psum->bf16 copy + DVE bf16 mult, or a
  single DVE scalar_tensor_tensor) instead of mult+add chains.
- g' = 1+tanh(z) with all 0.5 factors folded into scales; md_half =
  exp(-z)*(u+bu)/2 via the Act Exp table (same table set as Tanh).
- LayerNorm: bn_stats/bn_aggr on DVE, batched sqrt (Act) + reciprocal
  (DVE), apply as a single Act Identity op with per-token scale/bias APs,
  emitting fp8 token-major tiles for the PE transposes.
- MLP of each pair is deferred and interleaved into the NEXT pair's scan
  window so the PE never idles waiting for the serial scan chain.

Activation-table sets: sqrt (LN) / tanh+exp (gates) / gelu (MLP); copies
use Identity which lives in every set.  Program order batches same-set
ops to keep table loads to ~6 per kernel.
"""
import json
import os
import types

import numpy as np

import concourse.bass as bass
import concourse.mybir as mybir
from concourse.tile import TileContext
from concourse.bass_utils import run_bass_kernel_spmd

F32 = mybir.dt.float32
BF16 = mybir.dt.bfloat16
FP8 = mybir.dt.float8e4
AF = mybir.ActivationFunctionType
OP = mybir.AluOpType

B, H, W, C, T = 16, 32, 32, 384, 8
HID = 4 * C
EPS = 1e-6
NCORES = 8
BSH = B // NCORES              # samples per core
NTOK = BSH * H * W             # 2048 tokens per core
GTOK = 512                     # tokens per group
NG = NTOK // GTOK              # 4 groups
TPG = GTOK // 128              # 4 token-tiles per group
KT = C // 128                  # 3 channel tiles
MH = HID // 128                # 12 hidden tiles
WS = 64.0                      # fp8 weight pre-scale

# which scan chunks use the Act-copy path (others use a single DVE stt)
ACT_CHUNKS = tuple(
    int(c) for c in os.environ.get("KERNEL_ACT_CHUNKS", "1,2").split(",") if c != ""
)


# ---------------------------------------------------------------- bir fix --
# This container's walrus rejects instructions whose sync-wait list exceeds
# the opcode's wait slots (an SP Drain has none free).  Move excess waits
# onto EventSemaphore instructions inserted before the instruction on the
# same engine queue; waits still happen-before, so semantics are unchanged.
_WAIT_LIMITS = {"Drain": 0}
_WAIT_DEFAULT = 1


def _fix_bir_json(bj: bytes) -> bytes:
    bir = json.loads(bj)
    counter = [0]

    def fix_blocks(blocks):
        for b in blocks:
            insts = b.get("instructions")
            if insts:
                new = []
                for inst in insts:
                    si = inst.get("sync_info")
                    waits = (si or {}).get("on_wait") or []
                    limit = _WAIT_LIMITS.get(inst.get("opcode"), _WAIT_DEFAULT)
                    if len(waits) > limit:
                        n_extra = len(waits) - limit
                        extra, keep = waits[:n_extra], waits[n_extra:]
                        for wv in extra:
                            counter[0] += 1
                            new.append({
                                "name": f"I-wfix-{counter[0]}",
                                "opcode": "EventSemaphore",
                                "engine": inst["engine"],
                                "ins": [],
                                "outs": [],
                                "sync_info": {"on_update": [], "on_wait": [wv]},
                                "debug": inst.get("debug", 0),
                            })
                        si["on_wait"] = keep
                    new.append(inst)
                b["instructions"] = new
            fix_blocks(b.get("blocks") or [])

    for fn in bir.get("functions", []):
        fix_blocks(fn.get("blocks") or [])
    return json.dumps(bir).encode()


def _patch_nc(nc):
    orig = nc.to_json_bytes

    def to_json_bytes(self):
        return _fix_bir_json(orig())

    nc.to_json_bytes = types.MethodType(to_json_bytes, nc)
    return nc


# ----------------------------------------------------------- device build --
def build_nc(repeat=1):
    nc = bass.Bass()

    x_in = nc.declare_dram_parameter("x", [NTOK, C], F32, isOutput=False)
    wu_dr_d = nc.declare_dram_parameter("wu_dr", [128, 2, C], FP8, isOutput=False)
    wu2_d = nc.declare_dram_parameter("wu2", [128, C], FP8, isOutput=False)
    wg_dr_d = nc.declare_dram_parameter("wg_dr", [128, 2, C], FP8, isOutput=False)
    wg2_d = nc.declare_dram_parameter("wg2", [128, C], FP8, isOutput=False)
    a_d = nc.declare_dram_parameter("a", [C, C], BF16, isOutput=False)
    w1_dr_d = nc.declare_dram_parameter("w1_dr", [128, 2, HID], FP8, isOutput=False)
    w12_d = nc.declare_dram_parameter("w12", [128, HID], FP8, isOutput=False)
    w2_dr_d = nc.declare_dram_parameter("w2_dr", [128, MH // 2, 2, C], FP8,
                                        isOutput=False)
    # bias columns (f32): bu/2 (KT), bg (KT), -bg (KT), b1 (MH)
    NBIAS = 3 * KT + MH
    bias_d = nc.declare_dram_parameter("bias", [128, NBIAS], F32, isOutput=False)
    b2_d = nc.declare_dram_parameter("b2", [1, C], FP8, isOutput=False)
    eye8_d = nc.declare_dram_parameter("eye8", [128, 128], FP8, isOutput=False)
    eyeb_d = nc.declare_dram_parameter("eyeb", [128, 128], BF16, isOutput=False)
    eye2_d = nc.declare_dram_parameter("eye2", [128, 128], BF16, isOutput=False)
    ones_d = nc.declare_dram_parameter("ones", [1, 128], FP8, isOutput=False)
    out_d = nc.declare_dram_parameter("out", [NTOK, C], F32, isOutput=True)

    with TileContext(nc) as tc:
        with (
            tc.tile_pool(name="wp", bufs=1) as wp,
            tc.tile_pool(name="gp", bufs=2) as gp,
            tc.tile_pool(name="hidp", bufs=2) as hidp,
            tc.tile_pool(name="hp", bufs=2) as hp,
            tc.tile_pool(name="tp", bufs=3) as tp,
            tc.tile_pool(name="sp", bufs=4) as sp,
            tc.tile_pool(name="ps", bufs=6, space="PSUM") as ps,
            tc.tile_pool(name="pst", bufs=2, space="PSUM") as pst,
        ):
            # ---- weights / constants (loaded once) ----
            wu_dr = wp.tile([128, 2, C], FP8, tag="wu_dr")
            wu2_t = wp.tile([128, C], FP8, tag="wu2")
            wg_dr = wp.tile([128, 2, C], FP8, tag="wg_dr")
            wg2_t = wp.tile([128, C], FP8, tag="wg2")
            a_t = [wp.tile([128, C], BF16, tag=f"a{k}", name=f"a{k}") for k in range(KT)]
            w1_dr = wp.tile([128, 2, HID], FP8, tag="w1_dr")
            w12_t = wp.tile([128, HID], FP8, tag="w12")
            w2_dr = wp.tile([128, MH // 2, 2, C], FP8, tag="w2_dr")
            bias_t = wp.tile([128, NBIAS], F32, tag="bias")
            b2_t = wp.tile([1, C], FP8, tag="b2")
            eye8_t = wp.tile([128, 128], FP8, tag="eye8")
            eyeb_t = wp.tile([128, 128], BF16, tag="eyeb")
            eye2_t = wp.tile([128, 128], BF16, tag="eye2")
            ones_t = wp.tile([1, 128], FP8, tag="ones")
            eps_t = wp.tile([128, 1], F32, tag="eps")
            nc.vector.memset(eps_t, EPS)
            nc.sync.dma_start(out=eye8_t, in_=eye8_d[:, :])
            nc.sync.dma_start(out=eyeb_t, in_=eyeb_d[:, :])
            nc.sync.dma_start(out=eye2_t, in_=eye2_d[:, :])
            nc.sync.dma_start(out=bias_t, in_=bias_d[:, :])

            def bcol(i):
                return bias_t[:, i:i + 1]

            def load_mid_weights():
                nc.sync.dma_start(out=wu_dr, in_=wu_dr_d[:, :, :])
                nc.sync.dma_start(out=wu2_t, in_=wu2_d[:, :])
                nc.sync.dma_start(out=wg_dr, in_=wg_dr_d[:, :, :])
                nc.sync.dma_start(out=wg2_t, in_=wg2_d[:, :])
                for k in range(KT):
                    s = slice(k * 128, (k + 1) * 128)
                    nc.sync.dma_start(out=a_t[k], in_=a_d[s, :])

            def load_late_weights():
                nc.sync.dma_start(out=w1_dr, in_=w1_dr_d[:, :, :])
                nc.sync.dma_start(out=w12_t, in_=w12_d[:, :])
                nc.sync.dma_start(out=w2_dr, in_=w2_dr_d[:, :, :, :])
                nc.sync.dma_start(out=b2_t, in_=b2_d[:, :])
                nc.sync.dma_start(out=ones_t, in_=ones_d[:, :])

            def norm_to_cm(grp, src_tm, xcm, pref):
                """LN stats (DVE) + batched rstd + Act apply -> fp8 cm tiles."""
                mvg = sp.tile([128, TPG, 2], F32, tag="mvg",
                              name=f"{pref}mvg{grp}")
                for it in range(TPG):
                    mv6 = sp.tile([128, 6], F32, tag="mv6")
                    nc.vector.bn_stats(out=mv6, in_=src_tm[:, it, :])
                    nc.vector.bn_aggr(out=mvg[:, it, :], in_=mv6)
                sd = sp.tile([128, TPG], F32, tag="sd", name=f"{pref}sd{grp}")
                nc.scalar.activation(out=sd, in_=mvg[:, :, 1],
                                     func=AF.Sqrt, bias=eps_t, scale=1.0)
                rr = sp.tile([128, TPG], F32, tag="rr", name=f"{pref}rr{grp}")
                nc.vector.reciprocal(out=rr, in_=sd)
                mur = sp.tile([128, TPG], F32, tag="mur", name=f"{pref}mur{grp}")
                nc.vector.scalar_tensor_tensor(
                    out=mur, in0=mvg[:, :, 0], scalar=-1.0, in1=rr,
                    op0=OP.mult, op1=OP.mult)
                for it in range(TPG):
                    xnb = tp.tile([128, C], BF16, tag="xnb", bufs=2)
                    nc.scalar.activation(out=xnb, in_=src_tm[:, it, :],
                                         func=AF.Identity,
                                         bias=mur[:, it:it + 1],
                                         scale=rr[:, it:it + 1])
                    ptn = pst.tile([128, KT, 128], BF16, tag="pt")
                    for c in range(KT):
                        nc.tensor.transpose(ptn[:, c, :],
                                            xnb[:, c * 128:(c + 1) * 128],
                                            eyeb_t)
                    dst = xcm[:, :, it * 128:(it + 1) * 128]
                    if it % 2 == 0:
                        nc.scalar.activation(out=dst, in_=ptn,
                                             func=AF.Identity)
                    else:
                        nc.vector.tensor_copy(out=dst, in_=ptn)

            def phase_a(grp):
                """load x + norm1 -> fp8 channel-major xn"""
                st = {}
                st["x_tm"] = x_tm = gp.tile([128, TPG, C], F32, tag="x_tm",
                                            name=f"x_tm{grp}", bufs=3)
                for it in range(TPG):
                    row0 = (grp * TPG + it) * 128
                    nc.sync.dma_start(out=x_tm[:, it, :],
                                      in_=x_in[row0:row0 + 128, :])
                st["xn_cm"] = xcm = gp.tile([128, KT, GTOK], FP8,
                                            tag="xn_cm", name=f"xn_cm{grp}")
                norm_to_cm(grp, x_tm, xcm, "a")
                return st

            def phase_b(grp, st):
                """u/g projections -> g' = 1+tanh(z), md_half, h1"""
                xcm = st["xn_cm"]
                st["gp"] = gph = gp.tile([128, KT, GTOK], BF16, tag="gph",
                                         name=f"gph{grp}")
                st["md"] = mdh = gp.tile([128, KT, GTOK], BF16, tag="mdh",
                                         name=f"mdh{grp}")
                h1 = hp.tile([128, KT, GTOK], BF16, tag=f"h{grp % 2}",
                             name=f"h1_{grp}")
                for m in range(KT):
                    msl = slice(m * 128, (m + 1) * 128)
                    psu = ps.tile([128, GTOK], F32, tag="ps")
                    psg = ps.tile([128, GTOK], F32, tag="ps")
                    nc.tensor.matmul(psu, wu_dr[:, :, msl], xcm[:, 0:2, :],
                                     start=True, stop=False,
                                     perf_mode=mybir.MatmulPerfMode.DoubleRow)
                    nc.tensor.matmul(psu, wu2_t[:, msl], xcm[:, 2, :],
                                     start=False, stop=True)
                    nc.tensor.matmul(psg, wg_dr[:, :, msl], xcm[:, 0:2, :],
                                     start=True, stop=False,
                                     perf_mode=mybir.MatmulPerfMode.DoubleRow)
                    nc.tensor.matmul(psg, wg2_t[:, msl], xcm[:, 2, :],
                                     start=False, stop=True)
                    # ub2 = u_b/2   (psu = 64*u; bias col bu/2)
                    ub2 = tp.tile([128, GTOK], BF16, tag="ub2", bufs=2)
                    nc.scalar.activation(out=ub2, in_=psu, func=AF.Identity,
                                         bias=bcol(m), scale=1.0 / (2 * WS))
                    th = tp.tile([128, GTOK], BF16, tag="th", bufs=2)
                    nc.scalar.activation(out=th, in_=psg, func=AF.Tanh,
                                         bias=bcol(KT + m), scale=1.0 / (2 * WS))
                    et = tp.tile([128, GTOK], BF16, tag="et", bufs=2)
                    nc.scalar.activation(out=et, in_=psg, func=AF.Exp,
                                         bias=bcol(2 * KT + m), scale=-1.0 / WS)
                    nc.vector.tensor_scalar(out=gph[:, m, :], in0=th,
                                            scalar1=1.0, scalar2=None,
                                            op0=OP.add)
                    nc.vector.tensor_mul(out=mdh[:, m, :], in0=et, in1=ub2)
                    nc.vector.tensor_mul(out=h1[:, m, :], in0=gph[:, m, :],
                                         in1=mdh[:, m, :])
                st["h"] = h1

            def scan_step(grp, st, last=False):
                gph, mdh, h_prev = st["gp"], st["md"], st["h"]
                h_next = hp.tile([128, KT, GTOK], BF16,
                                 tag=f"h{grp % 2}", name=f"h{grp}")
                for m in range(KT):
                    msl = slice(m * 128, (m + 1) * 128)
                    psh = ps.tile([128, GTOK], F32, tag="ps")
                    nc.tensor.matmul(psh, eye2_t, mdh[:, m, :],
                                     start=True, stop=False)
                    for k in range(KT):
                        nc.tensor.matmul(psh, a_t[k][:, msl], h_prev[:, k, :],
                                         start=False, stop=(k == KT - 1))
                    if m in ACT_CHUNKS:
                        cc = tp.tile([128, GTOK], BF16, tag="cc", bufs=3)
                        nc.scalar.activation(out=cc, in_=psh,
                                             func=AF.Identity, scale=0.5)
                        nc.vector.tensor_mul(out=h_next[:, m, :],
                                             in0=cc, in1=gph[:, m, :])
                    else:
                        nc.vector.scalar_tensor_tensor(
                            out=h_next[:, m, :], in0=psh, scalar=0.5,
                            in1=gph[:, m, :], op0=OP.mult, op1=OP.mult)
                st["h"] = h_next

            def residual1(grp, st):
                """x2 = x + h^T  (token-major, f32)"""
                h_last, x_tm = st["h"], st["x_tm"]
                st["x2_tm"] = x2_tm = gp.tile([128, TPG, C], F32, tag="x2_tm",
                                              name=f"x2_tm{grp}")
                for it in range(TPG):
                    ptb = pst.tile([128, KT, 128], BF16, tag="pt")
                    for c in range(KT):
                        nc.tensor.transpose(
                            ptb[:, c, :],
                            h_last[:, c, it * 128:(it + 1) * 128], eyeb_t)
                    nc.vector.tensor_add(
                        out=x2_tm[:, it, :].rearrange("p (c q) -> p c q", c=KT),
                        in0=x_tm[:, it, :].rearrange("p (c q) -> p c q", c=KT),
                        in1=ptb)

            def norm2(grp, st):
                st["xn2_cm"] = xcm2 = gp.tile([128, KT, GTOK], FP8,
                                              tag="xn2_cm", name=f"xn2_cm{grp}")
                norm_to_cm(grp, st["x2_tm"], xcm2, "n")

            def mlp_units(grp, st):
                """Generator of fine-grained mlp work units for interleaving."""
                xcm2, x2_tm = st["xn2_cm"], st["x2_tm"]
                hid_t = hidp.tile([128, MH, GTOK], FP8, tag="hid",
                                  name=f"hid{grp}")

                def f_unit(mh):
                    def run():
                        msl = slice(mh * 128, (mh + 1) * 128)
                        psh = ps.tile([128, GTOK], F32, tag="ps")
                        nc.tensor.matmul(psh, w1_dr[:, :, msl], xcm2[:, 0:2, :],
                                         start=True, stop=False,
                                         perf_mode=mybir.MatmulPerfMode.DoubleRow)
                        nc.tensor.matmul(psh, w12_t[:, msl], xcm2[:, 2, :],
                                         start=False, stop=True)
                        nc.scalar.activation(
                            out=hid_t[:, mh, :], in_=psh,
                            func=AF.Gelu_apprx_tanh,
                            bias=bcol(3 * KT + mh), scale=1.0 / WS)
                    return run

                def g_unit(it):
                    def run():
                        tsl = slice(it * 128, (it + 1) * 128)
                        pso = ps.tile([128, C], F32, tag="ps")
                        for j in range(MH // 2):
                            nc.tensor.matmul(
                                pso, hid_t[:, 2 * j:2 * j + 2, tsl],
                                w2_dr[:, j, :, :], start=(j == 0), stop=False,
                                perf_mode=mybir.MatmulPerfMode.DoubleRow)
                        nc.tensor.matmul(pso, ones_t, b2_t,
                                         start=False, stop=True)
                        nc.vector.scalar_tensor_tensor(
                            out=x2_tm[:, it, :], in0=pso, scalar=1.0 / WS,
                            in1=x2_tm[:, it, :], op0=OP.mult, op1=OP.add)
                        row0 = (grp * TPG + it) * 128
                        nc.sync.dma_start(out=out_d[row0:row0 + 128, :],
                                          in_=x2_tm[:, it, :])
                    return run

                return [f_unit(mh) for mh in range(MH)] + \
                       [g_unit(it) for it in range(TPG)]

            # ---------------- schedule ----------------
            npair = (NG // 2) * repeat
            pending = []          # deferred mlp units from the previous pair
            for pair_i in range(npair):
                pair = pair_i % (NG // 2)
                g0, g1 = 2 * pair, 2 * pair + 1
                s0 = phase_a(g0)
                s1 = phase_a(g1)
                if pair_i == 0:
                    load_mid_weights()
                phase_b(g0, s0)
                phase_b(g1, s1)
                if pair_i == 0:
                    load_late_weights()
                # scan with deferred-mlp interleave
                nunits = len(pending)
                per_step = -(-nunits // (T - 1)) if nunits else 0
                ui = 0
                for t in range(T - 1):
                    last = t == T - 2
                    scan_step(g0, s0, last)
                    scan_step(g1, s1, last)
                    for _ in range(per_step):
                        if ui < nunits:
                            pending[ui]()
                            ui += 1
                while ui < nunits:
                    pending[ui]()
                    ui += 1
                pending = []
                residual1(g0, s0)
                residual1(g1, s1)
                norm2(g0, s0)
                norm2(g1, s1)
                pending = mlp_units(g0, s0) + mlp_units(g1, s1)
                if pair_i == npair - 1:
                    for u in pending:
                        u()
                    pending = []
    return nc


_NC_CACHE = {}


def _get_nc():
    if "nc" not in _NC_CACHE:
        _NC_CACHE["nc"] = _patch_nc(build_nc())
    return _NC_CACHE["nc"]


# ---------------------------------------------------------------- kernel --
def kernel(x, norm1_scale, norm1_bias, Wu, bu, Wg, bg, A,
           norm2_scale, norm2_bias, mlp_w1, mlp_b1, mlp_w2, mlp_b2,
           _return_raw=False):
    import ml_dtypes
    E4 = ml_dtypes.float8_e4m3
    BF = ml_dtypes.bfloat16
    f = np.float32
    x = np.asarray(x, f)
    norm1_scale = np.asarray(norm1_scale, f)
    norm1_bias = np.asarray(norm1_bias, f)
    Wu, bu = np.asarray(Wu, f), np.asarray(bu, f)
    Wg, bg = np.asarray(Wg, f), np.asarray(bg, f)
    A = np.asarray(A, f)
    norm2_scale = np.asarray(norm2_scale, f)
    norm2_bias = np.asarray(norm2_bias, f)
    mlp_w1, mlp_b1 = np.asarray(mlp_w1, f), np.asarray(mlp_b1, f)
    mlp_w2, mlp_b2 = np.asarray(mlp_w2, f), np.asarray(mlp_b2, f)

    # fold LN affine into downstream weights
    wu = norm1_scale[:, None] * Wu
    bu_f = bu + norm1_bias @ Wu
    wg = norm1_scale[:, None] * Wg
    bg_f = bg + norm1_bias @ Wg
    w1 = norm2_scale[:, None] * mlp_w1
    b1_f = mlp_b1 + norm2_bias @ mlp_w1

    wu8 = np.ascontiguousarray(wu * WS).astype(E4)
    wg8 = np.ascontiguousarray(wg * WS).astype(E4)
    w18 = np.ascontiguousarray(w1 * WS).astype(E4)
    w28 = np.ascontiguousarray(mlp_w2 * WS).astype(E4)

    def drpack(w):
        # [256, X] -> [128, 2, X]: [p, i, :] = w[i*128 + p, :]
        return np.ascontiguousarray(w[:256].reshape(2, 128, -1).transpose(1, 0, 2))

    wu_drh = drpack(wu8)
    wg_drh = drpack(wg8)
    w1_drh = drpack(w18)
    # [1536, C] -> [128, 6, 2, C]
    w2_drh = np.ascontiguousarray(
        w28.reshape(MH // 2, 2, 128, C).transpose(2, 0, 1, 3))
    b28 = np.ascontiguousarray(mlp_b2[None, :] * WS).astype(E4)
    abf = np.ascontiguousarray(A).astype(BF)

    NBIAS = 3 * KT + MH
    biasp = np.empty((128, NBIAS), f)
    for m in range(KT):
        biasp[:, m] = 0.5 * bu_f[m * 128:(m + 1) * 128]
        biasp[:, KT + m] = 0.5 * bg_f[m * 128:(m + 1) * 128]
        biasp[:, 2 * KT + m] = -bg_f[m * 128:(m + 1) * 128]
    for m in range(MH):
        biasp[:, 3 * KT + m] = b1_f[m * 128:(m + 1) * 128]

    eye = np.eye(128, dtype=f)
    eye8 = eye.astype(E4)
    eyeb = eye.astype(BF)
    eye2 = (2.0 * eye).astype(BF)
    ones8 = np.ones((1, 128), f).astype(E4)

    xs = x.reshape(NCORES, NTOK, C)
    in_maps = [{
        "x": np.ascontiguousarray(xs[i]),
        "wu_dr": wu_drh, "wu2": np.ascontiguousarray(wu8[256:]),
        "wg_dr": wg_drh, "wg2": np.ascontiguousarray(wg8[256:]),
        "a": abf, "w1_dr": w1_drh, "w12": np.ascontiguousarray(w18[256:]),
        "w2_dr": w2_drh,
        "bias": biasp, "b2": b28,
        "eye8": eye8, "eyeb": eyeb, "eye2": eye2, "ones": ones8,
    } for i in range(NCORES)]

    res = run_bass_kernel_spmd(_get_nc(), in_maps, list(range(NCORES)))
    if _return_raw:
        return res
    out = np.concatenate([res.results[i]["out"] for i in range(NCORES)], axis=0)
    return out.reshape(B, H, W, C).astype(np.float32)
